# revision 3
# baseline (speedup 1.0000x reference)
"""PacConv2d (BlockPAC) Trainium2 kernel, v2.

Same math as v1 (see kernel.py) but restructured for less DMA and fewer
instructions:

  * non-center tap stack of x in fp8e4m3 (those taps carry exp(-0.5*||dg||^2)
    ~ exp(-16) weights; 6%% fp8 rounding contributes ~3e-4 output rel-err)
    -> 8.4MB instead of 16.8MB bf16.
  * guide tap stack gs stays bf16; the center stack gc is built ON DEVICE
    from a single 0.5MB center load via 8 SBUF->SBUF DMA copies
    (saves 3.7MB HBM).
  * center tap path in bf16 (k_center == 1 exactly): dominant term, bf16
    gives ~2e-3 norm rel-err overall, ~10x under the 2e-2 gate.
  * diff/square on GPSIMD (Pool) - the ACT engine only runs Exp and
    bias+Relu, DVE only the adaptive multiply.
  * ONE broadcast tensor_mul per row-block ([128, 4, 16, 128], e8
    stride-0-broadcast over the channel-group dim) instead of 4.
  * weight-outer matmul order (w0 over 4 chunks, ..., wc over 4 chunks)
    -> 6 ldweights per block instead of 22.

Per-core totals: ~19.6MB HBM + 4.2MB SBUF-to-SBUF DMA, ~350 instructions
(v1: 34MB, ~770).

Sharding: unchanged - one sample per core, 8 cores, no collectives.
"""

import sys

import numpy as np

sys.path.insert(0, "/opt/trn_rl_repo")

import ml_dtypes

from concourse import bass, mybir, tile

# ---------------------------------------------------------------- constants
B, CIN, COUT, CG, H, W = 8, 64, 64, 16, 128, 128
KS, PAD = 3, 1
HP, WP = H + 2 * PAD, W + 2 * PAD
NCORES = 8

R = 16                      # output rows per block
NBLK = H // R               # 8 blocks
HGRP = 8                    # rows per psum half
CH = 4                      # rows per matmul chunk (N = 512)

TAPS = [(p // 3, p % 3) for p in range(9) if p != 4]
NT = len(TAPS)
CTR_I, CTR_J = 1, 1

F32 = mybir.dt.float32
BF = mybir.dt.bfloat16
F8 = mybir.dt.float8e4
NPBF = ml_dtypes.bfloat16
NPF8 = ml_dtypes.float8_e4m3

import os

FLAG_NOS2S = os.environ.get("PAC2_NOS2S", "0") == "1"      # gc from host, no SBUF->SBUF DMA
FLAG_NOPACK = os.environ.get("PAC2_NOPACK", "0") == "1"    # per-half [64,..] out PSUM, no tile_position packing
FLAG_NOBCAST = os.environ.get("PAC2_NOBCAST", "0") == "1"  # per-group muls, no stride-0 broadcast
FLAG_PLAINOUT = os.environ.get("PAC2_PLAINOUT", "1") == "1"  # 2 plain out DMAs, no rearranged AP
FLAG_NOF8 = os.environ.get("PAC2_NOF8", "0") == "1"        # xstk in bf16
FLAG_NOGP = os.environ.get("PAC2_NOGP", "0") == "1"        # diff on DVE instead of gpsimd
FLAG_NOWOUTER = os.environ.get("PAC2_NOWOUTER", "0") == "1"  # chunk-wise accumulation groups
UNROLL = int(os.environ.get("PAC2_UNROLL", "1"))  # repeat body in-NEFF (timing)
NBF = int(os.environ.get("PAC2_BFGROUPS", "0"))    # channel groups stored bf16 (rest fp8)
OUTBF = os.environ.get("PAC2_OUTBF", "1") == "1"   # bf16 output store (host upcasts)
GPMUL = os.environ.get("PAC2_GPMUL", "1") == "1"   # last group's multiply on gpsimd

_cache = {}

_SKIP_SPLIT = {"InstCall", "InstUnconditionalBranch", "InstEventSemaphore"}


def _split_waits(nc):
    """Peel extra Tile sync-waits onto single-wait EventSemaphore nops
    (walrus carries one wait slot per instruction)."""
    nopctr = [0]
    scratch_id = max(int(k) for k in nc.m.ant_sem_names) + 1
    nc.m.ant_sem_names[str(scratch_id)] = ["waitnop_scratch"]

    def mk_nop(engine, wait):
        nopctr[0] += 1
        nop = mybir.InstEventSemaphore(name=f"I-waitnop-{nopctr[0]}", ins=[], outs=[])
        nop.engine = engine
        upd = mybir.SyncUpdate(
            sync_type="semaphore", id=scratch_id, ant_name="waitnop_scratch",
            update_mode="sem-add-imm", update_value=0, update_reg=None,
        )
        nop.sync_info = mybir.SyncInfo(on_wait=[wait], on_update=[upd])
        return nop

    for f in nc.m.functions:
        for blk in f.blocks:
            out = []
            for inst in blk.instructions:
                si = inst.sync_info
                if (si is not None and si.on_wait and len(si.on_wait) > 1
                        and type(inst).__name__ not in _SKIP_SPLIT):
                    waits = list(si.on_wait)
                    for w in waits[:-1]:
                        out.append(mk_nop(inst.engine, w))
                    inst.sync_info = mybir.SyncInfo(
                        on_wait=[waits[-1]], on_update=list(si.on_update))
                out.append(inst)
            blk.instructions[:] = out


# ---------------------------------------------------------------- bass build
def _blob_layout():
    """512B-aligned byte offsets of each packed input section."""
    xb = 2 if FLAG_NOF8 else 1
    nf8 = 4 - NBF
    secs = [
        ("xstk", NBLK * 128 * nf8 * R * W * xb),
        ("xstkb", NBLK * 128 * NBF * R * W * 2),
        ("gs", 128 * H * W * 2),
        ("gc", 128 * H * W * 2),
        ("xpb", CIN * HP * WP * 2),
        ("wstk", 4 * 128 * COUT * 2),
        ("wc", CIN * COUT * 2),
        ("lhsd", 128 * 128 * 2),
        ("bias", 128 * 4),
    ]
    offs, sizes, o = {}, {}, 0
    for k, n in secs:
        offs[k], sizes[k] = o, n
        o += (n + 511) // 512 * 512
    offs["total"] = o
    return offs, sizes


def _build_nc():
    nc = bass.Bass(
        "TRN2",
        target_bir_lowering=False,
        debug=False,
        enable_asserts=False,
        num_devices=NCORES,
    )

    XDT = BF if FLAG_NOF8 else F8
    # all inputs live in ONE dram blob: every extra PJRT argument costs
    # ~45us per call through the axon proxy, dwarfing the compute.
    offs, sizes = _blob_layout()
    blob_d = nc.dram_tensor("blob", [offs["total"]], mybir.dt.uint8,
                            kind="ExternalInput").ap()

    def bview(key, dt, pattern, **dims):
        return (blob_d[offs[key] : offs[key] + sizes[key]]
                .bitcast(dt).rearrange(pattern, **dims))

    NF8 = 4 - NBF
    xstk_d = bview("xstk", XDT, "(b p g r w) -> b p g r w",
                   b=NBLK, p=128, g=NF8, r=R, w=W) if NF8 else None
    xstkb_d = bview("xstkb", BF, "(b p g r w) -> b p g r w",
                    b=NBLK, p=128, g=NBF, r=R, w=W) if NBF else None
    gs_d = bview("gs", BF, "(p h w) -> p h w", p=128, h=H, w=W)
    gc_d = bview("gc", BF, "(p h w) -> p h w", p=128, h=H, w=W)
    xpb_d = bview("xpb", BF, "(p h w) -> p h w", p=CIN, h=HP, w=WP)
    wstk_d = bview("wstk", BF, "(g p o) -> g p o", g=4, p=128, o=COUT)
    wc_d = bview("wc", BF, "(p o) -> p o", p=CIN, o=COUT)
    lhsd_d = bview("lhsd", BF, "(p o) -> p o", p=128, o=128)
    bias_d = bview("bias", F32, "(p o) -> p o", p=128, o=1)
    ODT = BF if OUTBF else F32
    out_d = nc.dram_tensor("out", [COUT, H, W], ODT, kind="ExternalOutput").ap()

    with tile.TileContext(nc) as tc:
        import contextlib

        with contextlib.ExitStack() as ctx:
            import os as _os
            _bufs = _os.environ.get("PAC2_BUFS", "3,3,2,2,2,3").split(",")
            bx, bg, bd, be, by, bo = (int(v) for v in _bufs)
            cst = ctx.enter_context(tc.tile_pool(name="cst", bufs=1))
            xsp = ctx.enter_context(tc.tile_pool(name="xsp", bufs=bx))
            gsp = ctx.enter_context(tc.tile_pool(name="gsp", bufs=bg))
            dfp = ctx.enter_context(tc.tile_pool(name="dfp", bufs=bd))
            e8p = ctx.enter_context(tc.tile_pool(name="e8p", bufs=be))
            yp = ctx.enter_context(tc.tile_pool(name="yp", bufs=by))
            osp = ctx.enter_context(tc.tile_pool(name="osp", bufs=bo))
            psd = ctx.enter_context(
                tc.tile_pool(name="psd", bufs=1 if FLAG_NOPACK else 2, space="PSUM"))
            pso = ctx.enter_context(
                tc.tile_pool(name="pso", bufs=1 if FLAG_NOPACK else 2, space="PSUM"))

            # ---- constants / per-sample tiles
            # tiny constants + the guide-center replication go first on the
            # HW rings (high_priority): everything downstream waits on them,
            # and ring semaphores are ordered, so anything queued before them
            # would stall the first ldweights / every diff.
            with tc.high_priority():
                w4 = []
                for g in range(4):
                    wt = cst.tile([128, COUT], BF, name=f"w{g}")
                    nc.sync.dma_start(wt[:], wstk_d[g, :, :])
                    w4.append(wt)
                wc_t = cst.tile([CIN, COUT], BF, name="wc")
                nc.sync.dma_start(wc_t[:], wc_d[:])
                lhsd_t = cst.tile([128, 128], BF, name="lhsd")
                nc.sync.dma_start(lhsd_t[:], lhsd_d[:])
                # bias replicated on both 64-partition halves (the out PSUM
                # tile packs row-half h2 into partitions 64*h2 .. 64*h2+64)
                bias_t = cst.tile([128, 1], F32, name="bias")
                nc.sync.dma_start(bias_t[:], bias_d[:])

                # block 0's guide tap/center stacks + x tap stack: first
                # on the rings (gc is streamed per block, no persistent tile)
                gcb0 = gsp.tile([128, R, W], BF, name="gcb")
                nc.scalar.dma_start(gcb0[:], gc_d[:, 0:R, :])
                gsb0 = gsp.tile([128, R, W], BF, name="gsb")
                nc.sync.dma_start(gsb0[:], gs_d[:, 0:R, :])
                xs0 = None
                if 4 - NBF:
                    xs0 = xsp.tile([128, 4 - NBF, R, W], XDT, name="xs")
                    nc.sync.dma_start(xs0[:], xstk_d[0])

            # xpb rides the scalar ring (one issue op on the ACT queue);
            # it is first needed ~20us in, after the first tap-weight passes.
            xpb_t = cst.tile([CIN, HP, WP], BF, name="xpb")
            nc.scalar.dma_start(xpb_t[:], xpb_d[:])

            for b in range(NBLK * UNROLL):
                b = b % NBLK
                r0 = R * b

                # gs feeds the long diff->sq->D->exp chain; xs is only
                # needed at the multiply. Fetch gs first. (block 0 was
                # prefetched in the high-priority scope)
                if b == 0 and gsb0 is not None:
                    gsb, xs, gcb = gsb0, xs0, gcb0
                    gsb0 = xs0 = gcb0 = None
                else:
                    gcb = gsp.tile([128, R, W], BF, name="gcb")
                    nc.scalar.dma_start(gcb[:], gc_d[:, r0 : r0 + R, :])
                    gsb = gsp.tile([128, R, W], BF, name="gsb")
                    nc.sync.dma_start(gsb[:], gs_d[:, r0 : r0 + R, :])
                    xs = None
                    if NF8:
                        xs = xsp.tile([128, NF8, R, W], XDT, name="xs")
                        nc.sync.dma_start(xs[:], xstk_d[b])
                if NBF:
                    xsb = xsp.tile([128, NBF, R, W], BF, name="xsb")
                    nc.sync.dma_start(xsb[:], xstkb_d[b])

                dif = dfp.tile([128, R, W], BF, name="dif")
                # with GPMUL the Pool engine is busy with the tap multiply;
                # the (cheap, 2x-mode) subtract runs on DVE instead
                sub_eng = nc.vector if (FLAG_NOGP or GPMUL) else nc.gpsimd
                sub_eng.tensor_sub(dif[:], gsb[:], gcb[:])
                sq = dfp.tile([128, R, W], BF, name="sq")
                nc.scalar.square(sq[:], dif[:])

                e8 = e8p.tile([128, R, W], BF, name="e8")
                for h in range(R // HGRP):
                    hr = HGRP * h
                    dps = psd.tile([128, HGRP, W], F32, name="dps")
                    for q in range(HGRP // CH):
                        nc.tensor.matmul(
                            dps[:, CH * q : CH * (q + 1), :],
                            lhsd_t[:],
                            sq[:, hr + CH * q : hr + CH * (q + 1), :],
                            start=True,
                            stop=True,
                        )
                    nc.scalar.activation(
                        e8[:, hr : hr + HGRP, :],
                        dps[:],
                        mybir.ActivationFunctionType.Exp,
                    )

                # one multiply per 8-row half (per dtype part) so the tap
                # matmuls can start on half 0 while half 1's exp is in flight
                y = yp.tile([128, 4, R, W], BF, name="y")
                for h2 in range(R // HGRP):
                    hr = HGRP * h2
                    if FLAG_NOBCAST:
                        for g in range(4):
                            src_ = (xs[:, g] if g < NF8
                                    else xsb[:, g - NF8])
                            nc.vector.tensor_mul(
                                y[:, g, hr : hr + HGRP, :],
                                src_[:, hr : hr + HGRP, :],
                                e8[:, hr : hr + HGRP, :],
                            )
                    else:
                        ngp = 1 if (GPMUL and NF8 >= 2) else 0
                        ndve = NF8 - ngp
                        if ndve:
                            e8b = (e8[:, hr : hr + HGRP, :].unsqueeze(1)
                                   .broadcast_to([128, ndve, HGRP, W]))
                            nc.vector.tensor_mul(
                                y[:, 0:ndve, hr : hr + HGRP, :],
                                xs[:, 0:ndve, hr : hr + HGRP, :],
                                e8b,
                            )
                        if ngp:
                            nc.gpsimd.tensor_mul(
                                y[:, ndve, hr : hr + HGRP, :],
                                xs[:, ndve, hr : hr + HGRP, :],
                                e8[:, hr : hr + HGRP, :],
                            )
                        if NBF:
                            e8c = (e8[:, hr : hr + HGRP, :].unsqueeze(1)
                                   .broadcast_to([128, NBF, HGRP, W]))
                            nc.vector.tensor_mul(
                                y[:, NF8:4, hr : hr + HGRP, :],
                                xsb[:, :, hr : hr + HGRP, :],
                                e8c,
                            )

                # out PSUM: row-half h2 lives in partitions 64*h2..64*h2+64,
                # so relu runs full-width and PSUM stays within 2 banks.
                if FLAG_NOPACK:
                    ops_l = [pso.tile([COUT, HGRP, W], F32, name=f"ops{h}")
                             for h in range(2)]
                else:
                    ops_t = pso.tile([128, HGRP, W], F32, name="ops")

                def mm(wi, h2, q, first, last):
                    r = HGRP * h2 + CH * q
                    if FLAG_NOPACK:
                        dst = ops_l[h2][:, CH * q : CH * (q + 1), :]
                    else:
                        dst = ops_t[
                            COUT * h2 : COUT * (h2 + 1), CH * q : CH * (q + 1), :
                        ]
                    if wi < 4:
                        nc.tensor.matmul(
                            dst, w4[wi][:], y[:, wi, r : r + CH, :],
                            start=first, stop=last, skip_group_check=True)
                    else:
                        nc.tensor.matmul(
                            dst, wc_t[:],
                            xpb_t[:, CTR_I + r0 + r : CTR_I + r0 + r + CH,
                                  CTR_J : CTR_J + W],
                            start=first, stop=last, skip_group_check=True)

                if FLAG_NOWOUTER:
                    for h2 in range(2):
                        for q in range(HGRP // CH):
                            for wi in range(5):
                                mm(wi, h2, q, wi == 0, wi == 4)
                else:
                    for wi in range(5):
                        for h2 in range(2):
                            for q in range(HGRP // CH):
                                mm(wi, h2, q, wi == 0, wi == 4)

                if FLAG_NOPACK:
                    for h2 in range(2):
                        osb = osp.tile([COUT, HGRP, W], ODT, name=f"osb{h2}")
                        nc.scalar.activation(
                            osb[:], ops_l[h2][:],
                            mybir.ActivationFunctionType.Relu,
                            bias=bias_t[0:COUT, :])
                        nc.scalar.dma_start(
                            out_d[:, r0 + HGRP * h2 : r0 + HGRP * (h2 + 1), :],
                            osb[:])
                else:
                    osb = osp.tile([128, HGRP, W], ODT, name="osb")
                    nc.scalar.activation(
                        osb[:], ops_t[:],
                        mybir.ActivationFunctionType.Relu,
                        bias=bias_t[:])
                    if FLAG_PLAINOUT:
                        for h2 in range(2):
                            nc.scalar.dma_start(
                                out_d[:, r0 + HGRP * h2 : r0 + HGRP * (h2 + 1), :],
                                osb[COUT * h2 : COUT * (h2 + 1), :, :])
                    else:
                        od = out_d[:, r0 : r0 + R, :].rearrange(
                            "o (h2 r) w -> h2 o r w", h2=2)
                        nc.sync.dma_start(od, osb[:])

    _split_waits(nc)
    return nc


def _get_nc():
    if "nc" not in _cache:
        _cache["nc"] = _build_nc()
    return _cache["nc"]


# ---------------------------------------------------------------- host side
def _prep_inputs(x, guide, weight, bias):
    x = np.asarray(x, dtype=np.float32)
    guide = np.asarray(guide, dtype=np.float32)
    weight = np.asarray(weight, dtype=np.float32)
    bias = np.asarray(bias, dtype=np.float32)

    xp = np.pad(x, ((0, 0), (0, 0), (PAD, PAD), (PAD, PAD)))
    gp = np.pad(guide, ((0, 0), (0, 0), (PAD, PAD), (PAD, PAD)))

    xpb = xp.astype(NPBF)
    # master tap stack in bf16; the fp8 section is cast at pack time so
    # bf16-stored groups keep full bf16 precision
    xstk = np.empty((B, 128, 4, H, W), dtype=NPBF)
    gs = np.empty((B, 128, H, W), dtype=NPBF)
    for t, (ti, tj) in enumerate(TAPS):
        for g in range(4):
            xstk[:, 16 * t : 16 * t + 16, g] = xp[
                :, 16 * g : 16 * g + 16, ti : ti + H, tj : tj + W
            ]
        gs[:, 16 * t : 16 * t + 16] = gp[:, :, ti : ti + H, tj : tj + W]
    # block-major xstk: per-partition contiguous 8KB per (block, partition)
    xstk = np.ascontiguousarray(
        xstk.reshape(B, 128, 4, NBLK, R, W).transpose(0, 3, 1, 2, 4, 5)
    )
    gcin = np.tile(guide.astype(NPBF), (1, 8, 1, 1))

    wstk = np.zeros((4, 128, COUT), dtype=np.float32)
    for g in range(4):
        for t, (ti, tj) in enumerate(TAPS):
            wstk[g, 16 * t : 16 * t + 16, :] = weight[
                :, 16 * g : 16 * g + 16, ti, tj
            ].T
    wstk = wstk.astype(NPBF)
    wc = np.ascontiguousarray(weight[:, :, CTR_I, CTR_J].T).astype(NPBF)

    lhsd = np.zeros((128, 128), dtype=np.float32)
    for t in range(NT):
        lhsd[16 * t : 16 * t + 16, 16 * t : 16 * t + 16] = -0.5
    lhsd = lhsd.astype(NPBF)

    # bias pre-replicated to both 64-partition halves of the packed PSUM tile
    bias2 = np.tile(bias.reshape(COUT, 1).astype(np.float32), (2, 1))

    offs, sizes = _blob_layout()
    in_maps = []
    for i in range(NCORES):
        blob = np.zeros(offs["total"], dtype=np.uint8)

        def put(key, arr):
            b = np.ascontiguousarray(arr).view(np.uint8).ravel()
            assert b.nbytes == sizes[key], (key, b.nbytes, sizes[key])
            blob[offs[key] : offs[key] + b.nbytes] = b

        nf8 = 4 - NBF
        if nf8:
            sec = xstk[i][:, :, 0:nf8]
            put("xstk", sec if FLAG_NOF8 else sec.astype(NPF8))
        if NBF:
            put("xstkb", xstk[i][:, :, nf8:4])
        put("gs", gs[i])
        put("gc", gcin[i])
        put("xpb", xpb[i])
        put("wstk", wstk)
        put("wc", wc)
        put("lhsd", lhsd)
        put("bias", bias2)
        in_maps.append({"blob": blob})
    return in_maps


def _run(in_maps, trace=False, **kw):
    from concourse.bass_utils import run_bass_kernel_spmd

    nc = _get_nc()
    last = None
    for attempt in range(3):
        try:
            res = run_bass_kernel_spmd(
                nc, in_maps, list(range(NCORES)), trace=trace, **kw
            )
            break
        except Exception as e:
            last = e
            import time as _t

            _t.sleep(20 * (attempt + 1))
    else:
        raise last
    out = np.stack([res.results[i]["out"] for i in range(NCORES)], axis=0)
    return out.astype(np.float32), res


def kernel(x, guide, weight, bias):
    in_maps = _prep_inputs(x, guide, weight, bias)
    out, _ = _run(in_maps)
    return out


# revision 9
# speedup vs baseline: 2.2773x; 2.2773x over previous
"""PacConv2d (BlockPAC) Trainium2 kernel, v2.

Math (per sample): k[p,hw] = exp(-0.5*sum_cg(guide_tap_p - guide_center)^2);
out[o,hw] = relu(bias[o] + sum_{c,p} x_tap_p[c,hw]*k[p,hw]*w[o,c,p]).
Sharding: data-parallel over batch B=8, one sample per NeuronCore, no
collectives.

Key design points (vs the v1 baseline, ~4.7x faster as measured):

  * ALL inputs are packed into ONE dram blob per core (bitcast views):
    every extra PJRT argument costs ~45us/call through the axon proxy,
    which dominated the baseline's measured time (9 args -> ~400us/call
    of pure dispatch overhead).
  * non-center x tap stack in fp8e4m3 (those taps carry exp(-16)-ish
    kernel weights; fp8 rounding adds ~3e-4 output rel err): 8.4MB
    instead of 16.8MB bf16. Center tap path stays bf16 and exact (k=1).
  * guide-center stack host-replicated into the blob, streamed per block
    (0.5MB slices) so no startup serialization; out stored bf16 and
    upcast on host. ~17MB HBM/core vs 34MB in v1.
  * engine placement: diff on DVE (2x bf16 mode), square on ACT, the
    adaptive multiply split 3 groups on DVE + 1 group on GPSIMD (GPSIMD
    multiplies fp8 at full rate, DVE at half), exp + bias-relu on ACT.
  * out PSUM packs the two 8-row halves of a block into partitions 0-63 /
    64-127 (matmul tile_position), so relu is one full-width op and PSUM
    fits 8 banks with double buffering.
  * weight-outer matmul order (w0..w3,wc each over 4 N=512 chunks,
    interleaved PSUM accumulation groups) -> 6 ldweights per block.
  * block-major xstk HBM layout -> per-block DMA is 128 contiguous 8KB
    descriptors; ~55 DMAs and ~350 instructions total vs ~770 in v1.

Model: ~97us/core (TimelineSim and CoreSim agree); measured steady-state
~75-110us/iter (in-NEFF unrolled), per-call ~190-330us including launch
overhead vs ~1030us for v1.

Env knobs (defaults are the shipped config): PAC2_OUTBF, PAC2_GPMUL,
PAC2_BFGROUPS, PAC2_UNROLL (timing), PAC2_BUFS, plus PAC2_NO* bisect
fallbacks (NOS2S is vestigial - gc now always comes from the blob).
"""

import sys

import numpy as np

sys.path.insert(0, "/opt/trn_rl_repo")

import ml_dtypes

from concourse import bass, mybir, tile

# ---------------------------------------------------------------- constants
B, CIN, COUT, CG, H, W = 8, 64, 64, 16, 128, 128
KS, PAD = 3, 1
HP, WP = H + 2 * PAD, W + 2 * PAD
NCORES = 8

R = 16                      # output rows per block
NBLK = H // R               # 8 blocks
HGRP = 8                    # rows per psum half
CH = 4                      # rows per matmul chunk (N = 512)

TAPS = [(p // 3, p % 3) for p in range(9) if p != 4]
NT = len(TAPS)
CTR_I, CTR_J = 1, 1

F32 = mybir.dt.float32
BF = mybir.dt.bfloat16
F8 = mybir.dt.float8e4
NPBF = ml_dtypes.bfloat16
NPF8 = ml_dtypes.float8_e4m3

import os

FLAG_NOS2S = os.environ.get("PAC2_NOS2S", "0") == "1"      # gc from host, no SBUF->SBUF DMA
FLAG_NOPACK = os.environ.get("PAC2_NOPACK", "0") == "1"    # per-half [64,..] out PSUM, no tile_position packing
FLAG_NOBCAST = os.environ.get("PAC2_NOBCAST", "0") == "1"  # per-group muls, no stride-0 broadcast
FLAG_PLAINOUT = os.environ.get("PAC2_PLAINOUT", "1") == "1"  # 2 plain out DMAs, no rearranged AP
FLAG_NOF8 = os.environ.get("PAC2_NOF8", "0") == "1"        # xstk in bf16
FLAG_NOGP = os.environ.get("PAC2_NOGP", "0") == "1"        # diff on DVE instead of gpsimd
FLAG_NOWOUTER = os.environ.get("PAC2_NOWOUTER", "0") == "1"  # chunk-wise accumulation groups
UNROLL = int(os.environ.get("PAC2_UNROLL", "1"))  # repeat body in-NEFF (timing)
NBF = int(os.environ.get("PAC2_BFGROUPS", "0"))    # channel groups stored bf16 (rest fp8)
OUTBF = os.environ.get("PAC2_OUTBF", "1") == "1"   # bf16 output store (host upcasts)
GPMUL = os.environ.get("PAC2_GPMUL", "1") == "1"   # last group's multiply on gpsimd

_cache = {}

_SKIP_SPLIT = {"InstCall", "InstUnconditionalBranch", "InstEventSemaphore"}


def _split_waits(nc):
    """Peel extra Tile sync-waits onto single-wait EventSemaphore nops
    (walrus carries one wait slot per instruction)."""
    nopctr = [0]
    scratch_id = max(int(k) for k in nc.m.ant_sem_names) + 1
    nc.m.ant_sem_names[str(scratch_id)] = ["waitnop_scratch"]

    def mk_nop(engine, wait):
        nopctr[0] += 1
        nop = mybir.InstEventSemaphore(name=f"I-waitnop-{nopctr[0]}", ins=[], outs=[])
        nop.engine = engine
        upd = mybir.SyncUpdate(
            sync_type="semaphore", id=scratch_id, ant_name="waitnop_scratch",
            update_mode="sem-add-imm", update_value=0, update_reg=None,
        )
        nop.sync_info = mybir.SyncInfo(on_wait=[wait], on_update=[upd])
        return nop

    for f in nc.m.functions:
        for blk in f.blocks:
            out = []
            for inst in blk.instructions:
                si = inst.sync_info
                if (si is not None and si.on_wait and len(si.on_wait) > 1
                        and type(inst).__name__ not in _SKIP_SPLIT):
                    waits = list(si.on_wait)
                    for w in waits[:-1]:
                        out.append(mk_nop(inst.engine, w))
                    inst.sync_info = mybir.SyncInfo(
                        on_wait=[waits[-1]], on_update=list(si.on_update))
                out.append(inst)
            blk.instructions[:] = out


# ---------------------------------------------------------------- bass build
def _blob_layout():
    """512B-aligned byte offsets of each packed input section."""
    xb = 2 if FLAG_NOF8 else 1
    nf8 = 4 - NBF
    secs = [
        ("xstk", NBLK * 128 * nf8 * R * W * xb),
        ("xstkb", NBLK * 128 * NBF * R * W * 2),
        ("gs", 128 * H * W * 2),
        ("gc", 128 * H * W * 2),
        ("xpb", CIN * HP * WP * 2),
        ("wstk", 4 * 128 * COUT * 2),
        ("wc", CIN * COUT * 2),
        ("lhsd", 128 * 128 * 2),
        ("bias", 128 * 4),
    ]
    offs, sizes, o = {}, {}, 0
    for k, n in secs:
        offs[k], sizes[k] = o, n
        o += (n + 511) // 512 * 512
    offs["total"] = o
    return offs, sizes


def _build_nc():
    nc = bass.Bass(
        "TRN2",
        target_bir_lowering=False,
        debug=False,
        enable_asserts=False,
        num_devices=NCORES,
    )

    XDT = BF if FLAG_NOF8 else F8
    # all inputs live in ONE dram blob: every extra PJRT argument costs
    # ~45us per call through the axon proxy, dwarfing the compute.
    offs, sizes = _blob_layout()
    blob_d = nc.dram_tensor("blob", [offs["total"]], mybir.dt.uint8,
                            kind="ExternalInput").ap()

    def bview(key, dt, pattern, **dims):
        return (blob_d[offs[key] : offs[key] + sizes[key]]
                .bitcast(dt).rearrange(pattern, **dims))

    NF8 = 4 - NBF
    xstk_d = bview("xstk", XDT, "(b p g r w) -> b p g r w",
                   b=NBLK, p=128, g=NF8, r=R, w=W) if NF8 else None
    xstkb_d = bview("xstkb", BF, "(b p g r w) -> b p g r w",
                    b=NBLK, p=128, g=NBF, r=R, w=W) if NBF else None
    gs_d = bview("gs", BF, "(p h w) -> p h w", p=128, h=H, w=W)
    gc_d = bview("gc", BF, "(p h w) -> p h w", p=128, h=H, w=W)
    xpb_d = bview("xpb", BF, "(p h w) -> p h w", p=CIN, h=HP, w=WP)
    wstk_d = bview("wstk", BF, "(g p o) -> g p o", g=4, p=128, o=COUT)
    wc_d = bview("wc", BF, "(p o) -> p o", p=CIN, o=COUT)
    lhsd_d = bview("lhsd", BF, "(p o) -> p o", p=128, o=128)
    bias_d = bview("bias", F32, "(p o) -> p o", p=128, o=1)
    ODT = BF if OUTBF else F32
    out_d = nc.dram_tensor("out", [COUT, H, W], ODT, kind="ExternalOutput").ap()

    with tile.TileContext(nc) as tc:
        import contextlib

        with contextlib.ExitStack() as ctx:
            import os as _os
            _bufs = _os.environ.get("PAC2_BUFS", "3,3,2,2,2,3").split(",")
            bx, bg, bd, be, by, bo = (int(v) for v in _bufs)
            cst = ctx.enter_context(tc.tile_pool(name="cst", bufs=1))
            xsp = ctx.enter_context(tc.tile_pool(name="xsp", bufs=bx))
            gsp = ctx.enter_context(tc.tile_pool(name="gsp", bufs=bg))
            dfp = ctx.enter_context(tc.tile_pool(name="dfp", bufs=bd))
            e8p = ctx.enter_context(tc.tile_pool(name="e8p", bufs=be))
            yp = ctx.enter_context(tc.tile_pool(name="yp", bufs=by))
            osp = ctx.enter_context(tc.tile_pool(name="osp", bufs=bo))
            psd = ctx.enter_context(
                tc.tile_pool(name="psd", bufs=1 if FLAG_NOPACK else 2, space="PSUM"))
            pso = ctx.enter_context(
                tc.tile_pool(name="pso", bufs=1 if FLAG_NOPACK else 2, space="PSUM"))

            # ---- constants / per-sample tiles
            # tiny constants + the guide-center replication go first on the
            # HW rings (high_priority): everything downstream waits on them,
            # and ring semaphores are ordered, so anything queued before them
            # would stall the first ldweights / every diff.
            with tc.high_priority():
                w4 = []
                for g in range(4):
                    wt = cst.tile([128, COUT], BF, name=f"w{g}")
                    nc.sync.dma_start(wt[:], wstk_d[g, :, :])
                    w4.append(wt)
                wc_t = cst.tile([CIN, COUT], BF, name="wc")
                nc.sync.dma_start(wc_t[:], wc_d[:])
                lhsd_t = cst.tile([128, 128], BF, name="lhsd")
                nc.sync.dma_start(lhsd_t[:], lhsd_d[:])
                # bias replicated on both 64-partition halves (the out PSUM
                # tile packs row-half h2 into partitions 64*h2 .. 64*h2+64)
                bias_t = cst.tile([128, 1], F32, name="bias")
                nc.sync.dma_start(bias_t[:], bias_d[:])

                # block 0's guide tap/center stacks + x tap stack: first
                # on the rings (gc is streamed per block, no persistent tile)
                gcb0 = gsp.tile([128, R, W], BF, name="gcb")
                nc.scalar.dma_start(gcb0[:], gc_d[:, 0:R, :])
                gsb0 = gsp.tile([128, R, W], BF, name="gsb")
                nc.sync.dma_start(gsb0[:], gs_d[:, 0:R, :])
                xs0 = None
                if 4 - NBF:
                    xs0 = xsp.tile([128, 4 - NBF, R, W], XDT, name="xs")
                    nc.sync.dma_start(xs0[:], xstk_d[0])

            # xpb rides the scalar ring (one issue op on the ACT queue);
            # it is first needed ~20us in, after the first tap-weight passes.
            xpb_t = cst.tile([CIN, HP, WP], BF, name="xpb")
            nc.scalar.dma_start(xpb_t[:], xpb_d[:])

            for b in range(NBLK * UNROLL):
                b = b % NBLK
                r0 = R * b

                # gs feeds the long diff->sq->D->exp chain; xs is only
                # needed at the multiply. Fetch gs first. (block 0 was
                # prefetched in the high-priority scope)
                if b == 0 and gsb0 is not None:
                    gsb, xs, gcb = gsb0, xs0, gcb0
                    gsb0 = xs0 = gcb0 = None
                else:
                    gcb = gsp.tile([128, R, W], BF, name="gcb")
                    nc.scalar.dma_start(gcb[:], gc_d[:, r0 : r0 + R, :])
                    gsb = gsp.tile([128, R, W], BF, name="gsb")
                    nc.sync.dma_start(gsb[:], gs_d[:, r0 : r0 + R, :])
                    xs = None
                    if NF8:
                        xs = xsp.tile([128, NF8, R, W], XDT, name="xs")
                        nc.sync.dma_start(xs[:], xstk_d[b])
                if NBF:
                    xsb = xsp.tile([128, NBF, R, W], BF, name="xsb")
                    nc.sync.dma_start(xsb[:], xstkb_d[b])

                dif = dfp.tile([128, R, W], BF, name="dif")
                # with GPMUL the Pool engine is busy with the tap multiply;
                # the (cheap, 2x-mode) subtract runs on DVE instead
                sub_eng = nc.vector if (FLAG_NOGP or GPMUL) else nc.gpsimd
                sub_eng.tensor_sub(dif[:], gsb[:], gcb[:])
                sq = dfp.tile([128, R, W], BF, name="sq")
                nc.scalar.square(sq[:], dif[:])

                e8 = e8p.tile([128, R, W], BF, name="e8")
                for h in range(R // HGRP):
                    hr = HGRP * h
                    dps = psd.tile([128, HGRP, W], F32, name="dps")
                    for q in range(HGRP // CH):
                        nc.tensor.matmul(
                            dps[:, CH * q : CH * (q + 1), :],
                            lhsd_t[:],
                            sq[:, hr + CH * q : hr + CH * (q + 1), :],
                            start=True,
                            stop=True,
                        )
                    nc.scalar.activation(
                        e8[:, hr : hr + HGRP, :],
                        dps[:],
                        mybir.ActivationFunctionType.Exp,
                    )

                # one multiply per 8-row half (per dtype part) so the tap
                # matmuls can start on half 0 while half 1's exp is in flight
                y = yp.tile([128, 4, R, W], BF, name="y")
                for h2 in range(R // HGRP):
                    hr = HGRP * h2
                    if FLAG_NOBCAST:
                        for g in range(4):
                            src_ = (xs[:, g] if g < NF8
                                    else xsb[:, g - NF8])
                            nc.vector.tensor_mul(
                                y[:, g, hr : hr + HGRP, :],
                                src_[:, hr : hr + HGRP, :],
                                e8[:, hr : hr + HGRP, :],
                            )
                    else:
                        ngp = 1 if (GPMUL and NF8 >= 2) else 0
                        ndve = NF8 - ngp
                        if ndve:
                            e8b = (e8[:, hr : hr + HGRP, :].unsqueeze(1)
                                   .broadcast_to([128, ndve, HGRP, W]))
                            nc.vector.tensor_mul(
                                y[:, 0:ndve, hr : hr + HGRP, :],
                                xs[:, 0:ndve, hr : hr + HGRP, :],
                                e8b,
                            )
                        if ngp:
                            nc.gpsimd.tensor_mul(
                                y[:, ndve, hr : hr + HGRP, :],
                                xs[:, ndve, hr : hr + HGRP, :],
                                e8[:, hr : hr + HGRP, :],
                            )
                        if NBF:
                            e8c = (e8[:, hr : hr + HGRP, :].unsqueeze(1)
                                   .broadcast_to([128, NBF, HGRP, W]))
                            nc.vector.tensor_mul(
                                y[:, NF8:4, hr : hr + HGRP, :],
                                xsb[:, :, hr : hr + HGRP, :],
                                e8c,
                            )

                # out PSUM: row-half h2 lives in partitions 64*h2..64*h2+64,
                # so relu runs full-width and PSUM stays within 2 banks.
                if FLAG_NOPACK:
                    ops_l = [pso.tile([COUT, HGRP, W], F32, name=f"ops{h}")
                             for h in range(2)]
                else:
                    ops_t = pso.tile([128, HGRP, W], F32, name="ops")

                def mm(wi, h2, q, first, last):
                    r = HGRP * h2 + CH * q
                    if FLAG_NOPACK:
                        dst = ops_l[h2][:, CH * q : CH * (q + 1), :]
                    else:
                        dst = ops_t[
                            COUT * h2 : COUT * (h2 + 1), CH * q : CH * (q + 1), :
                        ]
                    if wi < 4:
                        nc.tensor.matmul(
                            dst, w4[wi][:], y[:, wi, r : r + CH, :],
                            start=first, stop=last, skip_group_check=True)
                    else:
                        nc.tensor.matmul(
                            dst, wc_t[:],
                            xpb_t[:, CTR_I + r0 + r : CTR_I + r0 + r + CH,
                                  CTR_J : CTR_J + W],
                            start=first, stop=last, skip_group_check=True)

                if FLAG_NOWOUTER:
                    for h2 in range(2):
                        for q in range(HGRP // CH):
                            for wi in range(5):
                                mm(wi, h2, q, wi == 0, wi == 4)
                else:
                    for wi in range(5):
                        for h2 in range(2):
                            for q in range(HGRP // CH):
                                mm(wi, h2, q, wi == 0, wi == 4)

                if FLAG_NOPACK:
                    for h2 in range(2):
                        osb = osp.tile([COUT, HGRP, W], ODT, name=f"osb{h2}")
                        nc.scalar.activation(
                            osb[:], ops_l[h2][:],
                            mybir.ActivationFunctionType.Relu,
                            bias=bias_t[0:COUT, :])
                        nc.scalar.dma_start(
                            out_d[:, r0 + HGRP * h2 : r0 + HGRP * (h2 + 1), :],
                            osb[:])
                else:
                    osb = osp.tile([128, HGRP, W], ODT, name="osb")
                    nc.scalar.activation(
                        osb[:], ops_t[:],
                        mybir.ActivationFunctionType.Relu,
                        bias=bias_t[:])
                    if FLAG_PLAINOUT:
                        for h2 in range(2):
                            nc.scalar.dma_start(
                                out_d[:, r0 + HGRP * h2 : r0 + HGRP * (h2 + 1), :],
                                osb[COUT * h2 : COUT * (h2 + 1), :, :])
                    else:
                        od = out_d[:, r0 : r0 + R, :].rearrange(
                            "o (h2 r) w -> h2 o r w", h2=2)
                        nc.sync.dma_start(od, osb[:])

    _split_waits(nc)
    return nc


def _get_nc():
    if "nc" not in _cache:
        _cache["nc"] = _build_nc()
    return _cache["nc"]


# ---------------------------------------------------------------- host side
def _prep_inputs(x, guide, weight, bias):
    x = np.asarray(x, dtype=np.float32)
    guide = np.asarray(guide, dtype=np.float32)
    weight = np.asarray(weight, dtype=np.float32)
    bias = np.asarray(bias, dtype=np.float32)

    xp = np.pad(x, ((0, 0), (0, 0), (PAD, PAD), (PAD, PAD)))
    gp = np.pad(guide, ((0, 0), (0, 0), (PAD, PAD), (PAD, PAD)))

    xpb = xp.astype(NPBF)
    # master tap stack in bf16; the fp8 section is cast at pack time so
    # bf16-stored groups keep full bf16 precision
    xstk = np.empty((B, 128, 4, H, W), dtype=NPBF)
    gs = np.empty((B, 128, H, W), dtype=NPBF)
    for t, (ti, tj) in enumerate(TAPS):
        for g in range(4):
            xstk[:, 16 * t : 16 * t + 16, g] = xp[
                :, 16 * g : 16 * g + 16, ti : ti + H, tj : tj + W
            ]
        gs[:, 16 * t : 16 * t + 16] = gp[:, :, ti : ti + H, tj : tj + W]
    # block-major xstk: per-partition contiguous 8KB per (block, partition)
    xstk = np.ascontiguousarray(
        xstk.reshape(B, 128, 4, NBLK, R, W).transpose(0, 3, 1, 2, 4, 5)
    )
    gcin = np.tile(guide.astype(NPBF), (1, 8, 1, 1))

    wstk = np.zeros((4, 128, COUT), dtype=np.float32)
    for g in range(4):
        for t, (ti, tj) in enumerate(TAPS):
            wstk[g, 16 * t : 16 * t + 16, :] = weight[
                :, 16 * g : 16 * g + 16, ti, tj
            ].T
    wstk = wstk.astype(NPBF)
    wc = np.ascontiguousarray(weight[:, :, CTR_I, CTR_J].T).astype(NPBF)

    lhsd = np.zeros((128, 128), dtype=np.float32)
    for t in range(NT):
        lhsd[16 * t : 16 * t + 16, 16 * t : 16 * t + 16] = -0.5
    lhsd = lhsd.astype(NPBF)

    # bias pre-replicated to both 64-partition halves of the packed PSUM tile
    bias2 = np.tile(bias.reshape(COUT, 1).astype(np.float32), (2, 1))

    offs, sizes = _blob_layout()
    in_maps = []
    for i in range(NCORES):
        blob = np.zeros(offs["total"], dtype=np.uint8)

        def put(key, arr):
            b = np.ascontiguousarray(arr).view(np.uint8).ravel()
            assert b.nbytes == sizes[key], (key, b.nbytes, sizes[key])
            blob[offs[key] : offs[key] + b.nbytes] = b

        nf8 = 4 - NBF
        if nf8:
            sec = xstk[i][:, :, 0:nf8]
            put("xstk", sec if FLAG_NOF8 else sec.astype(NPF8))
        if NBF:
            put("xstkb", xstk[i][:, :, nf8:4])
        put("gs", gs[i])
        put("gc", gcin[i])
        put("xpb", xpb[i])
        put("wstk", wstk)
        put("wc", wc)
        put("lhsd", lhsd)
        put("bias", bias2)
        in_maps.append({"blob": blob})
    return in_maps


def _run(in_maps, trace=False, **kw):
    from concourse.bass_utils import run_bass_kernel_spmd

    nc = _get_nc()
    last = None
    for attempt in range(3):
        try:
            res = run_bass_kernel_spmd(
                nc, in_maps, list(range(NCORES)), trace=trace, **kw
            )
            break
        except Exception as e:
            last = e
            import time as _t

            _t.sleep(20 * (attempt + 1))
    else:
        raise last
    out = np.stack([res.results[i]["out"] for i in range(NCORES)], axis=0)
    return out.astype(np.float32), res


def kernel(x, guide, weight, bias):
    in_maps = _prep_inputs(x, guide, weight, bias)
    out, _ = _run(in_maps)
    return out
